# revision 4
# baseline (speedup 1.0000x reference)
"""Trainium2 Bass kernel for block-tridiagonal whitening (AR(1) recurrence).

Math: w_t = (x_t - mean(x_t)) @ V0 - w_{t-1} @ (V1 @ V0),  w_{-1} = 0.

Host-side transforms:
  V0c = (I - 11^T/C) @ V0   (centering folded into V0)
  M   = -(V1 @ V0)          (combined recurrence matrix)
so  w_t = x_t @ V0c + w_{t-1} @ M.

||M||_2 ~ 0.05, so the recurrence forgets its past within a few steps: each
S-step time chunk is computed independently after a J-step warm-up from a
y-only state (error ~ ||M||^J) — all chunks of a group advance in lockstep.

Precision budget (gate is 2e-2 max-rel; this lands ~3e-3): x, V0c, the y/w
staging buffer, and the output all fp16; the scan state and M fp8e4 (the
correction is only ~||M|| ~ 5% of w, so fp8's 4% rounding contributes
~2.5e-3).  That buys:
  - y = x @ V0c in 2 fp16 passes (vs 6 with hi/lo splits),
  - the scan matmul as a single fp8 DoubleRow pass (K=256 in one go,
    0.5 cycles/row),
  - fp16 TensorE transposes (1 cycle/row vs 2 for fp32),
  - half the input and output DMA bytes.

Staging is plain t-major [128c, 2h, b, J+TG]: scan step i touches columns
{i + S*cc} as a 3-d strided AP (fine for DVE at 1 el/cycle), warm-up reads
the previous chunk's tail in place (no halo slot), and the output transpose
reads its 128-column stationary operand directly from staging — the s-major
gather copies of the previous design disappear entirely.

Engine constraints honored: GpSimd cannot touch PSUM, ScalarE cannot add two
tensors, so PSUM-consuming adds live on DVE and PSUM->SBUF copies on ScalarE
(plus DVE for the output-tile copies it has slack for).

Sharding: batch 64 -> 8 cores x 8 rows; parameters replicated.
"""

import sys

sys.path.insert(0, "/opt/trn_rl_repo")

import numpy as np
import ml_dtypes

B, T, C = 64, 2048, 256
NCORES = 8
BS = B // NCORES   # batch rows per core
S = 32             # scan chunk length
J = 4              # warm-up steps (||M||^J ~ 6e-6 relative)
NG = 2             # time groups (pipelined)
TG = T // NG       # time steps per group
CHG = TG // S      # chunks per group per batch row
NSTEP = S + J      # scan steps per group
NT_G = TG // 128   # output t-tiles per group per batch row
LT = 2             # t-tiles per output DMA
NTILES_B = T // 128
COLS = TG + J      # staging columns per group (J seed cols up front)


def _build_program():
    import concourse.bacc as bacc
    import concourse.mybir as mybir
    import concourse.tile as tile

    f32 = mybir.dt.float32
    f16 = mybir.dt.float16
    f8 = mybir.dt.float8e4
    DR = mybir.MatmulPerfMode.DoubleRow

    nc = bacc.Bacc("TRN2", target_bir_lowering=False, debug=False)

    xh_dram = nc.dram_tensor("xh", [BS, T, C], f16, kind="ExternalInput")
    w_dram = nc.dram_tensor("w", [BS, T, C], f16, kind="ExternalOutput")
    # weight quadrants: q[p, kh, mh, j] = W[kh*128 + p, mh*128 + j]
    vq_dram = nc.dram_tensor("vq", [128, 2, 2, 128], f16, kind="ExternalInput")
    mq_dram = nc.dram_tensor("mq8", [128, 2, 2, 128], f8, kind="ExternalInput")
    id_dram = nc.dram_tensor("ident", [128, 128], f16, kind="ExternalInput")

    w_r = w_dram.ap().rearrange("b (n p) c -> p (b n) c", p=128)

    with tile.TileContext(nc) as tc:
        with (
            tc.tile_pool(name="const", bufs=1) as cpool,
            tc.tile_pool(name="stage", bufs=1) as spool,
            tc.tile_pool(name="state", bufs=1) as stpool,
            tc.tile_pool(name="xload", bufs=6) as xpool,
            tc.tile_pool(name="wstore", bufs=6) as wpool,
            tc.tile_pool(name="py", bufs=4, space="PSUM") as py_pool,
            tc.tile_pool(name="ps", bufs=2, space="PSUM") as ps_pool,
            tc.tile_pool(name="pout", bufs=2, space="PSUM") as pout_pool,
        ):
            vq = cpool.tile([128, 2, 2, 128], f16)
            mq = cpool.tile([128, 2, 2, 128], f8)
            ident = cpool.tile([128, 128], f16)
            nc.sync.dma_start(vq[:], vq_dram.ap()[:])
            nc.sync.dma_start(mq[:], mq_dram.ap()[:])
            nc.sync.dma_start(ident[:], id_dram.ap()[:])

            xw = [spool.tile([128, 2, BS, COLS], f16, tag=f"xw{g}",
                             name=f"xw{g}") for g in range(NG)]
            # group 0 seeds: w_{-1} = 0
            nc.gpsimd.memset(xw[0][:][:, :, :, 0:J], 0.0)

            # fp8 scan-state ping-pong tiles
            sf = [[stpool.tile([128, 2, BS, CHG], f8, tag=f"sf{g}{k}",
                               name=f"sf{g}{k}") for k in range(2)]
                  for g in range(NG)]

            # ---- emission helpers ------------------------------------------
            def emit_x_dma(g, b):
                ht = xpool.tile([128, 2, TG], f16, tag="ht", name="ht")
                for kh in range(2):
                    nc.sync.dma_start(
                        ht[:, kh, :],
                        xh_dram.ap()[b, g * TG:(g + 1) * TG,
                                     kh * 128:(kh + 1) * 128],
                        transpose=True)
                return ht

            def emit_y_half(g, b, mh, ht):
                # y mh-half for all TG cols of row b; kh accumulated in PSUM
                pms = []
                for ch in range(TG // 512):
                    pm = py_pool.tile([128, 512], f32, tag="pmy", name="pmy")
                    for kh in range(2):
                        nc.tensor.matmul(
                            pm[:], vq[:, kh, mh, :],
                            ht[:, kh, ch * 512:(ch + 1) * 512],
                            start=(kh == 0), stop=(kh == 1))
                    pms.append(pm)
                for ch, pm in enumerate(pms):
                    nc.scalar.copy(
                        xw[g][:][:, mh, b, J + ch * 512:J + ch * 512 + 512],
                        pm[:])

            def emit_y_dup(b):
                # seed group 1 from group 0's last J y-columns
                nc.gpsimd.tensor_copy(
                    xw[1][:][:, :, b, 0:J], xw[0][:][:, :, b, TG:TG + J])

            def col_slice(g, i):
                # columns {i + S*cc} for cc in [0, CHG): time t = i - J + S*cc
                return xw[g][:][:, :, :, i:i + S * (CHG - 1) + 1:S]

            def emit_scan_step(g, i):
                if i == 0:
                    nc.vector.tensor_copy(sf[g][0][:], col_slice(g, 0))
                    return
                pm = ps_pool.tile([128, 2, BS, CHG], f32, tag="pms",
                                  name="pms")
                prev = sf[g][(i - 1) % 2]
                for mh in range(2):
                    nc.tensor.matmul(
                        pm[:, mh], mq[:, :, mh, :], prev[:],
                        start=True, stop=True, perf_mode=DR)
                ys = col_slice(g, i)
                # state first: it is the only thing the next step waits on
                if i < NSTEP - 1:
                    nc.vector.tensor_add(sf[g][i % 2][:], pm[:], ys)
                if i >= J:
                    nc.vector.tensor_add(ys, pm[:], ys)

            cp_alt = [0]

            def emit_tout(g, b, n0):
                wt = wpool.tile([128, LT, C], f16, tag="wt", name="wt")
                for l in range(LT):
                    t0 = (n0 + l) * 128
                    po = pout_pool.tile([128, C], f16, tag="po", name="po")
                    for h in range(2):
                        nc.tensor.transpose(
                            po[:, h * 128:(h + 1) * 128],
                            xw[g][:][:, h, b, J + t0:J + t0 + 128],
                            ident[:])
                    if cp_alt[0] % 2 == 0:
                        nc.vector.tensor_copy(wt[:, l, :], po[:])
                    else:
                        nc.scalar.copy(wt[:, l, :], po[:])
                    cp_alt[0] += 1
                idx = b * NTILES_B + g * NT_G + n0
                nc.sync.dma_start(w_r[:, idx:idx + LT, :], wt[:])

            # ---- emission schedule: software-pipelined phases --------------
            # 1. y(g0), transposing DMAs prefetched two rows ahead
            hts = {0: emit_x_dma(0, 0), 1: emit_x_dma(0, 1)}
            for b in range(BS):
                if b + 2 < BS:
                    hts[b + 2] = emit_x_dma(0, b + 2)
                for mh in range(2):
                    emit_y_half(0, b, mh, hts[b])
                emit_y_dup(b)
            # 2. scan(g0) woven with y(g1), one (b, mh) half every 2 steps
            y1q = [(b, mh) for b in range(BS) for mh in range(2)]
            hts1 = {0: emit_x_dma(1, 0), 1: emit_x_dma(1, 1)}
            qi = 0
            for i in range(NSTEP):
                emit_scan_step(0, i)
                if i % 2 == 0 and qi < len(y1q):
                    b, mh = y1q[qi]
                    qi += 1
                    if mh == 0 and b + 2 < BS:
                        hts1[b + 2] = emit_x_dma(1, b + 2)
                    emit_y_half(1, b, mh, hts1[b])
            while qi < len(y1q):
                b, mh = y1q[qi]
                qi += 1
                if mh == 0 and b + 2 < BS:
                    hts1[b + 2] = emit_x_dma(1, b + 2)
                emit_y_half(1, b, mh, hts1[b])
            # 3. scan(g1) woven with tout(g0)
            tout0 = [(b, n0) for b in range(BS) for n0 in range(0, NT_G, LT)]
            ti = 0
            for i in range(NSTEP):
                emit_scan_step(1, i)
                if ti < len(tout0):
                    emit_tout(0, *tout0[ti])
                    ti += 1
            for k in range(ti, len(tout0)):
                emit_tout(0, *tout0[k])
            # 4. tout(g1)
            for b in range(BS):
                for n0 in range(0, NT_G, LT):
                    emit_tout(1, b, n0)

    nc.compile()
    return nc


_NC_CACHE = None


def _prep_inputs(x, V_0, V_1):
    x = np.ascontiguousarray(np.asarray(x, dtype=np.float32))
    V0 = np.asarray(V_0, dtype=np.float64)
    V1 = np.asarray(V_1, dtype=np.float64)

    P = np.eye(C) - 1.0 / C
    V0c = (P @ V0).astype(np.float32)
    M = (-(V1 @ V0)).astype(np.float32)

    x_h = x.astype(np.float16)
    V_h = V0c.astype(np.float16)
    M_8 = M.astype(ml_dtypes.float8_e4m3)

    def quads(w):
        return np.ascontiguousarray(
            w.reshape(2, 128, 2, 128).transpose(1, 0, 2, 3))

    return x_h, quads(V_h), quads(M_8)


def kernel(x, V_0, V_1):
    global _NC_CACHE
    from concourse.bass_utils import run_bass_kernel_spmd

    x_h, vq, mq8 = _prep_inputs(x, V_0, V_1)
    ident = np.eye(128, dtype=np.float16)

    if _NC_CACHE is None:
        _NC_CACHE = _build_program()
    nc = _NC_CACHE

    in_maps = []
    for core in range(NCORES):
        sl = slice(core * BS, (core + 1) * BS)
        in_maps.append({
            "xh": np.ascontiguousarray(x_h[sl]),
            "vq": vq, "mq8": mq8, "ident": ident,
        })

    res = run_bass_kernel_spmd(nc, in_maps, core_ids=list(range(NCORES)))
    out = np.concatenate(
        [np.asarray(res.results[i]["w"]) for i in range(NCORES)], axis=0)
    return out.astype(np.float32)


# revision 9
# speedup vs baseline: 1.0580x; 1.0580x over previous
"""Trainium2 Bass kernel for block-tridiagonal whitening (AR(1) recurrence).

Math: w_t = (x_t - mean(x_t)) @ V0 - w_{t-1} @ (V1 @ V0),  w_{-1} = 0.

Host-side transforms:
  V0c = (I - 11^T/C) @ V0   (centering folded into V0)
  M   = -(V1 @ V0)          (combined recurrence matrix)
so  w_t = x_t @ V0c + w_{t-1} @ M.

||M||_2 ~ 0.05, so the recurrence forgets its past within a few steps: each
S-step time chunk runs independently after a J-step warm-up from a y-only
state (error ~ ||M||^J); all chunks of a group advance in lockstep.

Precision ladder (gate 2e-2 max-rel, this lands ~8e-3): x, V0c, staging and
output fp16; scan state + M fp8e4 (the correction is ~||M|| ~ 5% of w, so
fp8's ~4% rounding contributes ~2.5e-3).  That buys 2-pass fp16 y matmuls,
a single fp8 DoubleRow pass for the scan correction (K=256 in one go), fp16
TensorE transposes, and half the DMA bytes each way.

Hardware rules this design is shaped by (probed on-device):
  - Matmul/transpose operands allow ONE free dim -> the scan's scattered
    column sets cannot feed PE directly; s-major staging keeps them as
    contiguous 32-el runs for the DVE adds (690ns/512el vs 1.7us strided).
  - GpSimd cannot touch PSUM; ScalarE cannot add tensors -> PSUM-consuming
    adds live on DVE only.  Output adds are therefore emitted DEFERRED so
    they sit behind the next step's state add in DVE's in-order queue
    instead of stretching the scan's serial chain.
  - PSUM-side strides are cheap; SBUF-side inner strides are not.  The
    y copy (PSUM -> s-major staging) iterates chunk-inner: strided PSUM
    source, 16-el contiguous staging runs (423ns vs 2.5us the other way).
  - fp16 PSUM reads hit the DVE 2x mode (406ns/512el) -> transpose output
    copies are cheap on DVE.

Sharding: batch 64 -> 8 cores x 8 rows; parameters replicated.
"""

import sys

sys.path.insert(0, "/opt/trn_rl_repo")

import numpy as np
import ml_dtypes

B, T, C = 64, 2048, 256
NCORES = 8
BS = B // NCORES   # batch rows per core
S = 32             # scan chunk length
J = 4              # warm-up steps (||M||^J ~ 6e-6 relative)
HALO = 32          # reserved halo columns (only last J used)
NG = 2             # time groups (pipelined)
TG = T // NG       # time steps per group
CHG = TG // S      # chunks per group per batch row
NSTEP = S + J      # scan steps per group
LT = 2             # 128-row t-tiles per output DMA
COLS_PAD = 33 * 32 # s-major grid: position(t'') = (t''%32)*33 + t''//32
NTILES_B = T // 128


def _build_program():
    import concourse.bacc as bacc
    import concourse.mybir as mybir
    import concourse.tile as tile

    f32 = mybir.dt.float32
    f16 = mybir.dt.float16
    f8 = mybir.dt.float8e4
    DR = mybir.MatmulPerfMode.DoubleRow

    nc = bacc.Bacc("TRN2", target_bir_lowering=False, debug=False)

    xh_dram = nc.dram_tensor("xh", [BS, T, C], f16, kind="ExternalInput")
    w_dram = nc.dram_tensor("w", [BS, T, C], f16, kind="ExternalOutput")
    # weight quadrants: q[p, kh, mh, j] = W[kh*128 + p, mh*128 + j]
    vq_dram = nc.dram_tensor("vq", [128, 2, 2, 128], f16, kind="ExternalInput")
    mq_dram = nc.dram_tensor("mq8", [128, 2, 2, 128], f8, kind="ExternalInput")
    id_dram = nc.dram_tensor("ident", [128, 128], f16, kind="ExternalInput")

    w_r = w_dram.ap().rearrange("b (n p) c -> p (b n) c", p=128)

    with tile.TileContext(nc) as tc:
        with (
            tc.tile_pool(name="const", bufs=1) as cpool,
            tc.tile_pool(name="stage", bufs=1) as spool,
            tc.tile_pool(name="state", bufs=1) as stpool,
            tc.tile_pool(name="xload", bufs=6) as xpool,
            tc.tile_pool(name="wstore", bufs=6) as wpool,
            tc.tile_pool(name="tmp", bufs=6) as tpool,
            tc.tile_pool(name="py", bufs=3, space="PSUM") as py_pool,
            tc.tile_pool(name="ps", bufs=3, space="PSUM") as ps_pool,
            tc.tile_pool(name="pout", bufs=2, space="PSUM") as pout_pool,
        ):
            vq = cpool.tile([128, 2, 2, 128], f16)
            mq = cpool.tile([128, 2, 2, 128], f8)
            ident = cpool.tile([128, 128], f16)
            nc.sync.dma_start(vq[:], vq_dram.ap()[:])
            nc.sync.dma_start(mq[:], mq_dram.ap()[:])
            nc.sync.dma_start(ident[:], id_dram.ap()[:])

            xw = [spool.tile([128, 2, BS, COLS_PAD], f16, tag=f"xw{g}",
                             name=f"xw{g}") for g in range(NG)]
            # [cq, s] view of the s-major grid (memory: pos = s*33 + cq)
            xwq = [xw[g][:].rearrange("p h b (s cq) -> p h b cq s", cq=33)
                   for g in range(NG)]
            # zero the J used halo columns of group 0 (t'' in [28, 32))
            nc.gpsimd.memset(
                xw[0][:].rearrange(
                    "p h b (s cq) -> p h b s cq", cq=33)[
                        :, :, :, HALO - J:HALO, 0], 0.0)

            # fp8 scan-state ping-pong tiles
            sf = [[stpool.tile([128, 2, BS, CHG], f8, tag=f"sf{g}_{k}",
                               name=f"sf{g}_{k}") for k in range(2)]
                  for g in range(NG)]

            cp_y = [0]
            cp_t = [0]

            # ---- emission helpers ------------------------------------------
            def emit_x_dma(g, b):
                ht = xpool.tile([128, 2, TG], f16, tag="ht", name="ht")
                for kh in range(2):
                    nc.sync.dma_start(
                        ht[:, kh, :],
                        xh_dram.ap()[b, g * TG:(g + 1) * TG,
                                     kh * 128:(kh + 1) * 128],
                        transpose=True)
                return ht

            def emit_y_unit(g, b, mh, ch, ht):
                pm = py_pool.tile([128, 512], f32, tag="pmy", name="pmy")
                sl = slice(ch * 512, ch * 512 + 512)
                for kh in range(2):
                    nc.tensor.matmul(
                        pm[:], vq[:, kh, mh, :], ht[:, kh, sl],
                        start=(kh == 0), stop=(kh == 1))
                # t'' = HALO + ch*512 + u, u = a*32 + s -> dst pos
                # s*33 + (cq0 + a): iterate chunk-inner for 16-el dst runs
                cq0 = 1 + ch * 16
                dst = xwq[g][:, mh, b, cq0:cq0 + 16, :].rearrange(
                    "p cq s -> p s cq")
                src = pm[:].rearrange("p (a s) -> p s a", s=32)
                if cp_y[0] % 5 < 2:
                    nc.vector.tensor_copy(dst, src)
                else:
                    nc.scalar.copy(dst, src)
                cp_y[0] += 1

            def emit_y_dup(b):
                # seed group 1's halo from group 0's last J y-columns
                nc.gpsimd.tensor_copy(
                    xwq[1][:, :, b, 0, HALO - J:HALO],
                    xwq[0][:, :, b, 32, HALO - J:HALO])

            def col_slice(g, i):
                # columns {t'' = cc*32 + i + (HALO-J)} for cc in [0, CHG)
                tpp = i + HALO - J
                base = (tpp % 32) * 33 + tpp // 32
                return xw[g][:, :, :, base:base + CHG]

            def emit_scan_matmul_add(g, i):
                pm = ps_pool.tile([128, 2, BS, CHG], f32, tag="pms",
                                  name="pms")
                prev = sf[g][(i - 1) % 2]
                for mh in range(2):
                    nc.tensor.matmul(
                        pm[:, mh], mq[:, :, mh, :], prev[:],
                        start=True, stop=True, perf_mode=DR)
                ys = col_slice(g, i)
                # state add: the only op the next step waits on
                if i < NSTEP - 1:
                    nc.vector.tensor_add(sf[g][i % 2][:], pm[:], ys)
                return pm

            def emit_out_add(g, i, pm):
                if i >= J:
                    ys = col_slice(g, i)
                    nc.vector.tensor_add(ys, pm[:], ys)

            def emit_scan(g, weave):
                """Scan group g; weave[] = callables run once per step.

                Output adds are deferred by 2 steps so they queue on DVE
                behind the next state adds instead of inside the chain."""
                pend = []
                wi = 0
                for i in range(NSTEP):
                    if i == 0:
                        nc.vector.tensor_copy(sf[g][0][:], col_slice(g, 0))
                    else:
                        pm = emit_scan_matmul_add(g, i)
                        pend.append((i, pm))
                    if len(pend) > 1:
                        ii, ppm = pend.pop(0)
                        emit_out_add(g, ii, ppm)
                    if wi < len(weave):
                        weave[wi]()
                        wi += 1
                for ii, ppm in pend:
                    emit_out_add(g, ii, ppm)
                while wi < len(weave):
                    weave[wi]()
                    wi += 1

            def emit_tout(g, b, n0):
                """Gather (gpsimd) + fp16 transpose + one 512-el copy + DMA.

                The fp16 PSUM tile spans both t-tiles so the PSUM->SBUF
                copy is a single 512-el op in the DVE 2x fast path."""
                wt = wpool.tile([128, LT, C], f16, tag="wt", name="wt")
                po = pout_pool.tile([128, LT, C], f16, tag="po", name="po")
                for l in range(LT):
                    t0 = (n0 + l) * 128
                    cq0 = 1 + t0 // 32
                    tmp = tpool.tile([128, 2, 4, 32], f16, tag="tmp",
                                     name="tmp")
                    nc.gpsimd.tensor_copy(tmp[:],
                                          xwq[g][:, :, b, cq0:cq0 + 4, :])
                    tmpf = tmp[:].rearrange("p h a s -> p (h a s)")
                    for h in range(2):
                        nc.tensor.transpose(
                            po[:, l, h * 128:(h + 1) * 128],
                            tmpf[:, h * 128:(h + 1) * 128],
                            ident[:])
                if cp_t[0] % 2 == 0:
                    nc.vector.tensor_copy(wt[:], po[:])
                else:
                    nc.scalar.copy(wt[:], po[:])
                cp_t[0] += 1
                idx = b * NTILES_B + (g * TG) // 128 + n0
                nc.sync.dma_start(w_r[:, idx:idx + LT, :], wt[:])

            # ---- emission schedule: software-pipelined phases --------------
            # 1. y(g0), transposing DMAs prefetched two rows ahead
            hts = {0: emit_x_dma(0, 0), 1: emit_x_dma(0, 1)}
            for b in range(BS):
                if b + 2 < BS:
                    hts[b + 2] = emit_x_dma(0, b + 2)
                for mh in range(2):
                    for ch in range(TG // 512):
                        emit_y_unit(0, b, mh, ch, hts[b])
                emit_y_dup(b)

            # 2. scan(g0) with y(g1) woven in
            hts1 = {0: emit_x_dma(1, 0), 1: emit_x_dma(1, 1)}
            weave1 = []
            for b in range(BS):
                for mh in range(2):
                    for ch in range(TG // 512):
                        def _w(b=b, mh=mh, ch=ch):
                            if (mh, ch) == (0, 0) and b + 2 < BS:
                                hts1[b + 2] = emit_x_dma(1, b + 2)
                            emit_y_unit(1, b, mh, ch, hts1[b])
                        weave1.append(_w)
            emit_scan(0, weave1)

            # 3. scan(g1) with tout(g0) woven in
            weave2 = []
            for b in range(BS):
                for n0 in range(0, TG // 128, LT):
                    def _w(b=b, n0=n0):
                        emit_tout(0, b, n0)
                    weave2.append(_w)
            emit_scan(1, weave2)

            # 4. tout(g1)
            for b in range(BS):
                for n0 in range(0, TG // 128, LT):
                    emit_tout(1, b, n0)

    nc.compile()
    return nc


_NC_CACHE = None


def _prep_inputs(x, V_0, V_1):
    x = np.ascontiguousarray(np.asarray(x, dtype=np.float32))
    V0 = np.asarray(V_0, dtype=np.float64)
    V1 = np.asarray(V_1, dtype=np.float64)

    P = np.eye(C) - 1.0 / C
    V0c = (P @ V0).astype(np.float32)
    M = (-(V1 @ V0)).astype(np.float32)

    x_h = x.astype(np.float16)
    V_h = V0c.astype(np.float16)
    M_8 = M.astype(ml_dtypes.float8_e4m3)

    def quads(w):
        return np.ascontiguousarray(
            w.reshape(2, 128, 2, 128).transpose(1, 0, 2, 3))

    return x_h, quads(V_h), quads(M_8)


def kernel(x, V_0, V_1):
    global _NC_CACHE
    from concourse.bass_utils import run_bass_kernel_spmd

    x_h, vq, mq8 = _prep_inputs(x, V_0, V_1)
    ident = np.eye(128, dtype=np.float16)

    if _NC_CACHE is None:
        _NC_CACHE = _build_program()
    nc = _NC_CACHE

    in_maps = []
    for core in range(NCORES):
        sl = slice(core * BS, (core + 1) * BS)
        in_maps.append({
            "xh": np.ascontiguousarray(x_h[sl]),
            "vq": vq, "mq8": mq8, "ident": ident,
        })

    res = run_bass_kernel_spmd(nc, in_maps, core_ids=list(range(NCORES)))
    out = np.concatenate(
        [np.asarray(res.results[i]["w"]) for i in range(NCORES)], axis=0)
    return out.astype(np.float32)


# revision 15
# speedup vs baseline: 1.6552x; 1.5644x over previous
"""Trainium2 Bass kernel for block-tridiagonal whitening (AR(1) recurrence).

Math: w_t = (x_t - mean(x_t)) @ V0 - w_{t-1} @ (V1 @ V0),  w_{-1} = 0.

Host-side transforms:
  V0c = (I - 11^T/C) @ V0   (centering folded into V0)
  M   = -(V1 @ V0)          (combined recurrence matrix)
so  w_t = x_t @ V0c + w_{t-1} @ M.

||M||_2 ~ 0.05, so the recurrence forgets its past within a few steps: each
S-step time chunk runs independently after a J-step warm-up from a y-only
state (error ~ ||M||^J); all chunks of a group advance in lockstep.

Precision ladder (gate 2e-2 max-rel, this lands ~8e-3): x, V0c, staging and
output fp16; scan state + M fp8e4 (the correction is ~||M|| ~ 5% of w, so
fp8's ~4% rounding contributes ~2.5e-3).  That buys 2-pass fp16 y matmuls,
a single fp8 DoubleRow pass for the scan correction (K=256 in one go), fp16
TensorE transposes, and half the DMA bytes each way.

Hardware rules this design is shaped by (probed on-device):
  - Matmul/transpose operands allow ONE free dim -> the scan's scattered
    column sets cannot feed PE directly; s-major staging keeps them as
    contiguous 32-el runs for the DVE adds (690ns/512el vs 1.7us strided).
  - GpSimd cannot touch PSUM; ScalarE cannot add tensors -> PSUM-consuming
    adds live on DVE only.  Output adds are therefore emitted DEFERRED so
    they sit behind the next step's state add in DVE's in-order queue
    instead of stretching the scan's serial chain.
  - PSUM-side strides are cheap; SBUF-side inner strides are not.  The
    y copy (PSUM -> s-major staging) iterates chunk-inner: strided PSUM
    source, 16-el contiguous staging runs (423ns vs 2.5us the other way).
  - fp16 PSUM reads hit the DVE 2x mode (406ns/512el) -> transpose output
    copies are cheap on DVE.

Sharding: batch 64 -> 8 cores x 8 rows; parameters replicated.
"""

import sys

sys.path.insert(0, "/opt/trn_rl_repo")

import numpy as np
import ml_dtypes

B, T, C = 64, 2048, 256
NCORES = 8
BS = B // NCORES   # batch rows per core
S = 32             # scan chunk length
J = 4              # warm-up steps (||M||^J ~ 6e-6 relative)
HALO = 32          # reserved halo columns (only last J used)
NG = 2             # time groups (pipelined)
TG = T // NG       # time steps per group
CHG = TG // S      # chunks per group per batch row
NSTEP = S + J      # scan steps per group
LT = 2             # 128-row t-tiles per output DMA
COLS_PAD = 33 * 32 # s-major grid: position(t'') = (t''%32)*33 + t''//32
NTILES_B = T // 128


def _build_program():
    import concourse.bacc as bacc
    import concourse.mybir as mybir
    import concourse.tile as tile

    f32 = mybir.dt.float32
    f16 = mybir.dt.float16
    f8 = mybir.dt.float8e4
    DR = mybir.MatmulPerfMode.DoubleRow

    nc = bacc.Bacc("TRN2", target_bir_lowering=False, debug=False)

    xh_dram = nc.dram_tensor("xh", [BS, T, C], f16, kind="ExternalInput")
    # raw s-major staging dump; the host unscrambles to [B, T, C]
    w_dram = nc.dram_tensor("w", [NG, 128, 2, BS, COLS_PAD], f16,
                            kind="ExternalOutput")
    # weight quadrants: q[p, kh, mh, j] = W[kh*128 + p, mh*128 + j]
    vq_dram = nc.dram_tensor("vq", [128, 2, 2, 128], f16, kind="ExternalInput")
    mq_dram = nc.dram_tensor("mq8", [128, 2, 2, 128], f8, kind="ExternalInput")

    with tile.TileContext(nc) as tc:
        with (
            tc.tile_pool(name="const", bufs=1) as cpool,
            tc.tile_pool(name="stage", bufs=1) as spool,
            tc.tile_pool(name="state", bufs=1) as stpool,
            tc.tile_pool(name="xload", bufs=6) as xpool,
            tc.tile_pool(name="py", bufs=4, space="PSUM") as py_pool,
            tc.tile_pool(name="ps", bufs=4, space="PSUM") as ps_pool,
        ):
            vq = cpool.tile([128, 2, 2, 128], f16)
            mq = cpool.tile([128, 2, 2, 128], f8)
            nc.sync.dma_start(vq[:], vq_dram.ap()[:])
            nc.sync.dma_start(mq[:], mq_dram.ap()[:])

            xw = [spool.tile([128, 2, BS, COLS_PAD], f16, tag=f"xw{g}",
                             name=f"xw{g}") for g in range(NG)]
            # [cq, s] view of the s-major grid (memory: pos = s*33 + cq)
            xwq = [xw[g][:].rearrange("p h b (s cq) -> p h b cq s", cq=33)
                   for g in range(NG)]
            # zero the J used halo columns of group 0 (t'' in [28, 32))
            nc.gpsimd.memset(
                xw[0][:].rearrange(
                    "p h b (s cq) -> p h b s cq", cq=33)[
                        :, :, :, HALO - J:HALO, 0], 0.0)

            # fp8 scan-state ping-pong tiles
            sf = [[stpool.tile([128, 2, BS, CHG], f8, tag=f"sf{g}_{k}",
                               name=f"sf{g}_{k}") for k in range(2)]
                  for g in range(NG)]

            cp_y = [0]

            # ---- emission helpers ------------------------------------------
            def emit_x_dma(g, b):
                ht = xpool.tile([128, 2, TG], f16, tag="ht", name="ht")
                for kh in range(2):
                    nc.sync.dma_start(
                        ht[:, kh, :],
                        xh_dram.ap()[b, g * TG:(g + 1) * TG,
                                     kh * 128:(kh + 1) * 128],
                        transpose=True)
                return ht

            def emit_y_unit(g, b, mh, ch, ht):
                pm = py_pool.tile([128, 512], f32, tag="pmy", name="pmy")
                sl = slice(ch * 512, ch * 512 + 512)
                for kh in range(2):
                    nc.tensor.matmul(
                        pm[:], vq[:, kh, mh, :], ht[:, kh, sl],
                        start=(kh == 0), stop=(kh == 1))
                # t'' = HALO + ch*512 + u, u = a*32 + s -> dst pos
                # s*33 + (cq0 + a): iterate chunk-inner for 16-el dst runs
                cq0 = 1 + ch * 16
                dst = xwq[g][:, mh, b, cq0:cq0 + 16, :].rearrange(
                    "p cq s -> p s cq")
                src = pm[:].rearrange("p (a s) -> p s a", s=32)
                if cp_y[0] % 5 < 2:
                    nc.vector.tensor_copy(dst, src)
                else:
                    nc.scalar.copy(dst, src)
                cp_y[0] += 1

            def emit_y_dup(b):
                # seed group 1's halo from group 0's last J y-columns
                nc.gpsimd.tensor_copy(
                    xwq[1][:, :, b, 0, HALO - J:HALO],
                    xwq[0][:, :, b, 32, HALO - J:HALO])

            def col_slice(g, i):
                # columns {t'' = cc*32 + i + (HALO-J)} for cc in [0, CHG)
                tpp = i + HALO - J
                base = (tpp % 32) * 33 + tpp // 32
                return xw[g][:, :, :, base:base + CHG]

            def emit_scan_matmul_add(g, i):
                pm = ps_pool.tile([128, 2, BS, CHG], f32, tag="pms",
                                  name="pms")
                prev = sf[g][(i - 1) % 2]
                for mh in range(2):
                    nc.tensor.matmul(
                        pm[:, mh], mq[:, :, mh, :], prev[:],
                        start=True, stop=True, perf_mode=DR)
                ys = col_slice(g, i)
                # state add: the only op the next step waits on
                if i < NSTEP - 1:
                    nc.vector.tensor_add(sf[g][i % 2][:], pm[:], ys)
                return pm

            def emit_out_add(g, i, pm):
                if i >= J:
                    ys = col_slice(g, i)
                    nc.vector.tensor_add(ys, pm[:], ys)

            def emit_scan(g, weave):
                """Scan group g; weave[] = callables run once per step.

                Output adds are deferred by 2 steps so they queue on DVE
                behind the next state adds instead of inside the chain."""
                pend = []
                wi = 0
                for i in range(NSTEP):
                    if i == 0:
                        nc.vector.tensor_copy(sf[g][0][:], col_slice(g, 0))
                    else:
                        pm = emit_scan_matmul_add(g, i)
                        pend.append((i, pm))
                    if len(pend) > 1:
                        ii, ppm = pend.pop(0)
                        emit_out_add(g, ii, ppm)
                    if wi < len(weave):
                        weave[wi]()
                        wi += 1
                for ii, ppm in pend:
                    emit_out_add(g, ii, ppm)
                while wi < len(weave):
                    weave[wi]()
                    wi += 1

            def emit_dump(g, b):
                # raw staging dump; host unscrambles the s-major layout
                nc.sync.dma_start(w_dram.ap()[g, :, :, b, :],
                                  xw[g][:, :, b, :])

            # ---- emission schedule: software-pipelined phases --------------
            # 1. y(g0), transposing DMAs prefetched two rows ahead
            hts = {0: emit_x_dma(0, 0), 1: emit_x_dma(0, 1)}
            for b in range(BS):
                if b + 2 < BS:
                    hts[b + 2] = emit_x_dma(0, b + 2)
                for mh in range(2):
                    for ch in range(TG // 512):
                        emit_y_unit(0, b, mh, ch, hts[b])
                emit_y_dup(b)

            # 2. scan(g0) with y(g1) woven in
            hts1 = {0: emit_x_dma(1, 0), 1: emit_x_dma(1, 1)}
            weave1 = []
            for b in range(BS):
                for mh in range(2):
                    for ch in range(TG // 512):
                        def _w(b=b, mh=mh, ch=ch):
                            if (mh, ch) == (0, 0) and b + 2 < BS:
                                hts1[b + 2] = emit_x_dma(1, b + 2)
                            emit_y_unit(1, b, mh, ch, hts1[b])
                        weave1.append(_w)
            emit_scan(0, weave1)

            # 3. scan(g1) with dump(g0) woven in
            weave2 = [lambda b=b: emit_dump(0, b) for b in range(BS)]
            emit_scan(1, weave2)

            # 4. dump(g1)
            for b in range(BS):
                emit_dump(1, b)

    nc.compile()
    return nc


_NC_CACHE = None


def _prep_inputs(x, V_0, V_1):
    x = np.ascontiguousarray(np.asarray(x, dtype=np.float32))
    V0 = np.asarray(V_0, dtype=np.float64)
    V1 = np.asarray(V_1, dtype=np.float64)

    P = np.eye(C) - 1.0 / C
    V0c = (P @ V0).astype(np.float32)
    M = (-(V1 @ V0)).astype(np.float32)

    x_h = x.astype(np.float16)
    V_h = V0c.astype(np.float16)
    M_8 = M.astype(ml_dtypes.float8_e4m3)

    def quads(w):
        return np.ascontiguousarray(
            w.reshape(2, 128, 2, 128).transpose(1, 0, 2, 3))

    return x_h, quads(V_h), quads(M_8)


def _unscramble(dump):
    """[NG, 128, 2, BS, COLS_PAD] s-major staging dump -> [BS, T, C]."""
    tl = np.arange(TG)
    tpp = tl + HALO
    pos = (tpp % 32) * 33 + tpp // 32
    out = np.empty((BS, T, C), dtype=np.float16)
    for g in range(NG):
        sel = dump[g][:, :, :, pos]            # [128, 2, BS, TG]
        out[:, g * TG:(g + 1) * TG, :] = (
            sel.transpose(2, 3, 1, 0).reshape(BS, TG, C))
    return out


def kernel(x, V_0, V_1):
    global _NC_CACHE
    from concourse.bass_utils import run_bass_kernel_spmd

    x_h, vq, mq8 = _prep_inputs(x, V_0, V_1)

    if _NC_CACHE is None:
        _NC_CACHE = _build_program()
    nc = _NC_CACHE

    in_maps = []
    for core in range(NCORES):
        sl = slice(core * BS, (core + 1) * BS)
        in_maps.append({
            "xh": np.ascontiguousarray(x_h[sl]),
            "vq": vq, "mq8": mq8,
        })

    res = run_bass_kernel_spmd(nc, in_maps, core_ids=list(range(NCORES)))
    out = np.concatenate(
        [_unscramble(np.asarray(res.results[i]["w"]))
         for i in range(NCORES)], axis=0)
    return out.astype(np.float32)
